# revision 20
# baseline (speedup 1.0000x reference)
"""Trainium2 kernel for nn_MultiHeadClassifier.

Math: out[i] = W[task_labels[i]] @ x[i] + b[task_labels[i]]
  x [262144, 1024] f32, task_labels [262144] int, W [8, 32, 1024], b [8, 32]

Strategy (8 NeuronCores, task-parallel, fp8 x):
  - Host sorts rows by task; core c processes (up to NCAP=32768) rows of
    task c, so W[c] is a per-core constant and there is NO routing on
    device at all — each core runs a plain GEMM. The ~few hundred rows
    that overflow a core's capacity are computed on host (numpy) and the
    result is merged back; bias is added on host.
  - x is sent as fp8 e3m4 (1 byte: 4 mantissa bits), quartering the
    dominant HBM traffic vs f32. W is scaled by 128 and sent as an
    e3m4 hi+lo pair, which cancels the W quantization error to second
    order; the host divides the output by 128 (exact). Measured rel
    err 1.16e-2 vs the 2e-2 gate.
  - hi and lo live side by side in one M=64 stationary [128, 64], so
    each k-tile needs ONE matmul; the hi+lo reduction happens on the
    (otherwise idle) DVE as a fused PSUM add + bf16 cast. Two 512-row
    chunks pack into each PSUM bank via column tiling (positions 0/64),
    which the PE streams 2-way concurrently (the XBUS ceiling measured
    on HW) -> ~3.4us of PE per 2048-row quad vs ~5.2us of DMA.
  - x quads (2 MB, 16 KB/partition contiguous) stream on both HWDGE
    queues (SP/ACT alternating); wt is pre-transposed on host so its
    const DMA is a handful of contiguous descriptors (a host-side
    rearrange here costs 14 us of pipeline head). PSUM -> DVE add+cast
    to bf16 -> 512 KB output DMA per 4 quads (final group in 4 small
    pieces to cut the tail). Expected ~100-105 us.
"""

import sys

sys.path.insert(0, "/opt/trn_rl_repo")

import numpy as np
import ml_dtypes

import concourse.bass as bass
import concourse.tile as tile
from concourse import bacc, mybir
from concourse import bass_utils

B, D, C, T = 262144, 1024, 32, 8
NCORES = 8
P = 128
KO = D // P  # 8 contraction k-tiles
CH = 512  # rows per chunk (one PSUM column-tile)
QR = 2048  # rows per quad (one x DMA; 4 chunks in 2 PSUM banks)
NQ = 16  # quads per core
NCAP = QR * NQ  # 32768 rows per core capacity
GQ = 4  # quads per output DMA group
NG = NQ // GQ  # output groups
WSCALE = 128.0  # power of two; host divides the output by this exactly

F8 = ml_dtypes.float8_e3m4
F8LIM = 15.5  # max finite e3m4

# set by test harness to collect a profile; harness-invoked kernel() keeps it off
TRACE = False
LAST_RESULTS = None
LAST_IN_MAPS = None


def _build():
    f32 = mybir.dt.float32
    bf16 = mybir.dt.bfloat16
    f8 = mybir.dt.float8e3

    nc = bacc.Bacc("TRN2", debug=False, num_devices=NCORES)
    # xt[m, ki, ko, r]: one quad is a contiguous 2 MB region with
    # 16 KB contiguous per partition.
    xt_d = nc.dram_tensor("xt", [NQ, P, KO, QR], f8, kind="ExternalInput")
    # wt[ki, ko, 0:32]=hi, [ki, ko, 32:64]=lo (host-transposed, scaled)
    wt_d = nc.dram_tensor("wt", [P, KO, 2 * C], f8, kind="ExternalInput")
    out_d = nc.dram_tensor("out", [NG, P, GQ * CH], bf16, kind="ExternalOutput")

    with tile.TileContext(nc) as tc:
        with (
            tc.tile_pool(name="consts", bufs=1) as consts,
            tc.tile_pool(name="xpool", bufs=6) as xpool,
            tc.tile_pool(name="opool", bufs=3) as opool,
            tc.tile_pool(name="psum", bufs=6, space="PSUM") as psum,
        ):
            # first x quad in flight before the consts
            xq0 = xpool.tile([P, KO, QR], f8, tag="xq")
            nc.sync.dma_start(xq0[:], xt_d[0])

            # consts on the ACT ring (contiguous layout: cheap descriptors)
            wt = consts.tile([P, KO, 2 * C], f8)
            nc.scalar.dma_start(wt[:], wt_d[:])

            # Engine warmups: give PE and DVE one instruction that observes
            # the const DMA lane so steady-state instructions carry at most
            # one sync wait each.
            scratch = psum.tile([P, CH], f32, tag="y4")
            nc.tensor.matmul(
                scratch[:2, :2], wt[:, 0, :2], wt[:, 0, :2], start=True, stop=True
            )
            dve_scr = consts.tile([1, 2 * C], bf16)
            nc.vector.tensor_copy(dve_scr[:], wt[:1, 0, :])

            for m in range(NQ):
                g, q = m // GQ, m % GQ
                last_g = g == NG - 1
                if m == 0:
                    xq = xq0
                else:
                    xq = xpool.tile([P, KO, QR], f8, tag="xq")
                    # stripe the x stream across both HWDGE queues
                    eng = nc.sync if m % 2 == 0 else nc.scalar
                    eng.dma_start(xq[:], xt_d[m])
                if q == 0 and not last_g:
                    out_g = opool.tile([P, GQ * CH], bf16, tag="out")
                # 2 banks x 2 column positions = 4 chunks of 512 rows
                ya = psum.tile([P, CH], f32, tag="y4")
                yb = psum.tile([P, CH], f32, tag="y4")
                for ko in range(KO):
                    for bank, (j0, j1) in ((ya, (0, 1)), (yb, (2, 3))):
                        for pos, j in ((0, j0), (64, j1)):
                            nc.tensor.matmul(
                                bank[pos : pos + 2 * C, :],
                                wt[:, ko, :],
                                xq[:, ko, CH * j : CH * (j + 1)],
                                start=(ko == 0),
                                stop=(ko == KO - 1),
                                tile_position=(0, pos),
                                skip_group_check=True,
                            )
                if last_g:
                    # final group: 4 small pieces so the tail DMA after the
                    # last matmul is ~128 KB, not 512 KB
                    piece = opool.tile([P, CH], bf16, tag="piece")
                    dst = piece
                    off = 0
                else:
                    dst = out_g
                    off = CH * q
                for j in range(4):
                    bank = ya if j < 2 else yb
                    base = 64 * (j % 2)
                    sl = dst[C * j : C * (j + 1), off : off + CH]
                    # out = hi + lo: ACT copies hi w/ bf16 cast, DVE adds lo
                    # (an instruction may read at most one PSUM operand)
                    nc.scalar.copy(sl, bank[base : base + C, :])
                    nc.vector.tensor_tensor(
                        sl, sl, bank[base + C : base + 2 * C, :], mybir.AluOpType.add
                    )
                if last_g:
                    nc.scalar.dma_start(out_d[g, :, CH * q : CH * (q + 1)], piece[:])
                elif q == GQ - 1:
                    nc.scalar.dma_start(out_d[g], out_g[:])
    nc.compile()
    return nc


_NC = None


def _get_nc():
    global _NC
    if _NC is None:
        _NC = _build()
    return _NC


def kernel(x, task_labels, W, b):
    global LAST_RESULTS, LAST_IN_MAPS
    x = np.asarray(x)
    if x.dtype != np.float32:
        x = x.astype(np.float32)
    labels = np.asarray(task_labels).astype(np.int64)
    W = np.asarray(W)
    if W.dtype != np.float32:
        W = W.astype(np.float32)
    b = np.asarray(b)
    if b.dtype != np.float32:
        b = b.astype(np.float32)

    order = np.argsort(labels, kind="stable")  # rows grouped by task
    counts = np.bincount(labels, minlength=T)
    starts = np.concatenate([[0], np.cumsum(counts)])

    in_maps = []
    over_rows = []  # (task, global row indices beyond capacity)
    for t in range(T):
        seg_idx = order[starts[t] : starts[t + 1]]
        n_dev = min(counts[t], NCAP)
        xs = np.zeros((NCAP, D), dtype=F8)
        xs[:n_dev] = x[seg_idx[:n_dev]]
        # xt[m, ki, ko, r] = xs[m*QR + r, ko*P + ki]
        xt = np.ascontiguousarray(xs.reshape(NQ, QR, KO, P).transpose(0, 3, 2, 1))
        ws = W[t].astype(np.float64) * WSCALE
        hi = np.clip(ws, -F8LIM, F8LIM).astype(F8)
        lo = np.clip(ws - hi.astype(np.float64), -F8LIM, F8LIM).astype(F8)
        # wt[ki, ko, 0:32]=hi[c, ko*128+ki], [ki, ko, 32:64]=lo
        wt = np.empty((P, KO, 2 * C), dtype=F8)
        wt[:, :, :C] = hi.T.reshape(KO, P, C).transpose(1, 0, 2)
        wt[:, :, C:] = lo.T.reshape(KO, P, C).transpose(1, 0, 2)
        in_maps.append({"xt": xt, "wt": np.ascontiguousarray(wt)})
        if counts[t] > NCAP:
            over_rows.append((t, seg_idx[NCAP:]))

    LAST_IN_MAPS = in_maps
    nc = _get_nc()
    res = bass_utils.run_bass_kernel_spmd(
        nc, in_maps, core_ids=list(range(NCORES)), trace=TRACE
    )
    LAST_RESULTS = res

    out = np.empty((B, C), dtype=np.float32)
    inv_scale = np.float32(1.0 / WSCALE)
    for t in range(T):
        seg_idx = order[starts[t] : starts[t + 1]]
        n_dev = min(counts[t], NCAP)
        # out_d[g, 32j+c, 512q+r] -> row 2048*(4g+q) + 512*j + r
        o = np.asarray(res.results[t]["out"]).astype(np.float32)
        o = o.reshape(NG, 4, C, GQ, CH).transpose(0, 3, 1, 4, 2).reshape(NCAP, C)
        out[seg_idx[:n_dev]] = o[:n_dev]
    out *= inv_scale
    for t, idx in over_rows:
        out[idx] = x[idx] @ W[t].T
    out += b[labels]
    return out


# revision 21
# speedup vs baseline: 1.0695x; 1.0695x over previous
"""Trainium2 kernel for nn_MultiHeadClassifier.

Math: out[i] = W[task_labels[i]] @ x[i] + b[task_labels[i]]
  x [262144, 1024] f32, task_labels [262144] int, W [8, 32, 1024], b [8, 32]

Strategy (8 NeuronCores, task-parallel, fp8 x):
  - Host sorts rows by task; core c processes (up to NCAP=32768) rows of
    task c, so W[c] is a per-core constant and there is NO routing on
    device at all — each core runs a plain GEMM. The ~few hundred rows
    that overflow a core's capacity are computed on host (numpy) and the
    result is merged back; bias is added on host.
  - x is sent as fp8 e3m4 (1 byte: 4 mantissa bits), quartering the
    dominant HBM traffic vs f32. W is scaled by 128 and sent as an
    e3m4 hi+lo pair, which cancels the W quantization error to second
    order; the host divides the output by 128 (exact). Measured rel
    err 1.16e-2 vs the 2e-2 gate.
  - hi and lo live side by side in one M=64 stationary [128, 64], so
    each k-tile needs ONE matmul; the hi+lo reduction happens on the
    (otherwise idle) DVE as a fused PSUM add + bf16 cast. Two 512-row
    chunks pack into each PSUM bank via column tiling (positions 0/64),
    which the PE streams 2-way concurrently (the XBUS ceiling measured
    on HW) -> ~3.4us of PE per 2048-row quad vs ~5.2us of DMA.
  - x quads (2 MB, 16 KB/partition contiguous) stream on both HWDGE
    queues (SP/ACT alternating); wt is pre-transposed on host so its
    const DMA is a handful of contiguous descriptors (a host-side
    rearrange here costs 14 us of pipeline head). PSUM -> DVE add+cast
    to bf16 -> 512 KB output DMA per 4 quads (final group in 4 small
    pieces to cut the tail). Expected ~100-105 us.
"""

import sys

sys.path.insert(0, "/opt/trn_rl_repo")

import numpy as np
import ml_dtypes

import concourse.bass as bass
import concourse.tile as tile
from concourse import bacc, mybir
from concourse import bass_utils

B, D, C, T = 262144, 1024, 32, 8
NCORES = 8
P = 128
KO = D // P  # 8 contraction k-tiles
CH = 512  # rows per chunk (one PSUM column-tile)
QR = 2048  # rows per quad (one x DMA; 4 chunks in 2 PSUM banks)
NQ = 16  # quads per core
NCAP = QR * NQ  # 32768 rows per core capacity
GQ = 4  # quads per output DMA group
NG = NQ // GQ  # output groups
WSCALE = 128.0  # power of two; host divides the output by this exactly

F8 = ml_dtypes.float8_e3m4
F8LIM = 15.5  # max finite e3m4

# set by test harness to collect a profile; harness-invoked kernel() keeps it off
TRACE = False
LAST_RESULTS = None
LAST_IN_MAPS = None


def _build():
    f32 = mybir.dt.float32
    bf16 = mybir.dt.bfloat16
    f8 = mybir.dt.float8e3

    nc = bacc.Bacc("TRN2", debug=False, num_devices=NCORES)
    # xt[m, ki, ko, r]: one quad is a contiguous 2 MB region with
    # 16 KB contiguous per partition.
    xt_d = nc.dram_tensor("xt", [NQ, P, KO, QR], f8, kind="ExternalInput")
    # wt[ki, ko, 0:32]=hi, [ki, ko, 32:64]=lo (host-transposed, scaled)
    wt_d = nc.dram_tensor("wt", [P, KO, 2 * C], f8, kind="ExternalInput")
    out_d = nc.dram_tensor("out", [NG, P, GQ * CH], bf16, kind="ExternalOutput")

    with tile.TileContext(nc) as tc:
        with (
            tc.tile_pool(name="consts", bufs=1) as consts,
            tc.tile_pool(name="xpool", bufs=6) as xpool,
            tc.tile_pool(name="opool", bufs=3) as opool,
            tc.tile_pool(name="psum", bufs=6, space="PSUM") as psum,
        ):
            # first x quad in flight before the consts
            xq0 = xpool.tile([P, KO, QR], f8, tag="xq")
            nc.sync.dma_start(xq0[:], xt_d[0])

            # consts on the ACT ring (contiguous layout: cheap descriptors)
            wt = consts.tile([P, KO, 2 * C], f8)
            nc.scalar.dma_start(wt[:], wt_d[:])

            # Engine warmups: give PE and DVE one instruction that observes
            # the const DMA lane so steady-state instructions carry at most
            # one sync wait each.
            scratch = psum.tile([P, CH], f32, tag="y4")
            nc.tensor.matmul(
                scratch[:2, :2], wt[:, 0, :2], wt[:, 0, :2], start=True, stop=True
            )
            dve_scr = consts.tile([1, 2 * C], bf16)
            nc.vector.tensor_copy(dve_scr[:], wt[:1, 0, :])

            for m in range(NQ):
                g, q = m // GQ, m % GQ
                last_g = g == NG - 1
                if m == 0:
                    xq = xq0
                else:
                    xq = xpool.tile([P, KO, QR], f8, tag="xq")
                    # all x on the SP ring: the ACT sequencer is busy with
                    # hi-copies, and x triggers must never queue behind them
                    nc.sync.dma_start(xq[:], xt_d[m])
                if q == 0 and not last_g:
                    out_g = opool.tile([P, GQ * CH], bf16, tag="out")
                # 2 banks x 2 column positions = 4 chunks of 512 rows
                ya = psum.tile([P, CH], f32, tag="y4")
                yb = psum.tile([P, CH], f32, tag="y4")
                for ko in range(KO):
                    for bank, (j0, j1) in ((ya, (0, 1)), (yb, (2, 3))):
                        for pos, j in ((0, j0), (64, j1)):
                            nc.tensor.matmul(
                                bank[pos : pos + 2 * C, :],
                                wt[:, ko, :],
                                xq[:, ko, CH * j : CH * (j + 1)],
                                start=(ko == 0),
                                stop=(ko == KO - 1),
                                tile_position=(0, pos),
                                skip_group_check=True,
                            )
                if last_g:
                    # final group: 4 small pieces so the tail DMA after the
                    # last matmul is ~128 KB, not 512 KB
                    piece = opool.tile([P, CH], bf16, tag="piece")
                    dst = piece
                    off = 0
                else:
                    dst = out_g
                    off = CH * q
                for j in range(4):
                    bank = ya if j < 2 else yb
                    base = 64 * (j % 2)
                    sl = dst[C * j : C * (j + 1), off : off + CH]
                    # out = hi + lo: ACT copies hi w/ bf16 cast, DVE adds lo
                    # (an instruction may read at most one PSUM operand)
                    nc.scalar.copy(sl, bank[base : base + C, :])
                    nc.vector.tensor_tensor(
                        sl, sl, bank[base + C : base + 2 * C, :], mybir.AluOpType.add
                    )
                if last_g:
                    nc.scalar.dma_start(out_d[g, :, CH * q : CH * (q + 1)], piece[:])
                elif q == GQ - 1:
                    nc.scalar.dma_start(out_d[g], out_g[:])
    nc.compile()
    return nc


_NC = None


def _get_nc():
    global _NC
    if _NC is None:
        _NC = _build()
    return _NC


def kernel(x, task_labels, W, b):
    global LAST_RESULTS, LAST_IN_MAPS
    x = np.asarray(x)
    if x.dtype != np.float32:
        x = x.astype(np.float32)
    labels = np.asarray(task_labels).astype(np.int64)
    W = np.asarray(W)
    if W.dtype != np.float32:
        W = W.astype(np.float32)
    b = np.asarray(b)
    if b.dtype != np.float32:
        b = b.astype(np.float32)

    order = np.argsort(labels, kind="stable")  # rows grouped by task
    counts = np.bincount(labels, minlength=T)
    starts = np.concatenate([[0], np.cumsum(counts)])

    in_maps = []
    over_rows = []  # (task, global row indices beyond capacity)
    for t in range(T):
        seg_idx = order[starts[t] : starts[t + 1]]
        n_dev = min(counts[t], NCAP)
        xs = np.zeros((NCAP, D), dtype=F8)
        xs[:n_dev] = x[seg_idx[:n_dev]]
        # xt[m, ki, ko, r] = xs[m*QR + r, ko*P + ki]
        xt = np.ascontiguousarray(xs.reshape(NQ, QR, KO, P).transpose(0, 3, 2, 1))
        ws = W[t].astype(np.float64) * WSCALE
        hi = np.clip(ws, -F8LIM, F8LIM).astype(F8)
        lo = np.clip(ws - hi.astype(np.float64), -F8LIM, F8LIM).astype(F8)
        # wt[ki, ko, 0:32]=hi[c, ko*128+ki], [ki, ko, 32:64]=lo
        wt = np.empty((P, KO, 2 * C), dtype=F8)
        wt[:, :, :C] = hi.T.reshape(KO, P, C).transpose(1, 0, 2)
        wt[:, :, C:] = lo.T.reshape(KO, P, C).transpose(1, 0, 2)
        in_maps.append({"xt": xt, "wt": np.ascontiguousarray(wt)})
        if counts[t] > NCAP:
            over_rows.append((t, seg_idx[NCAP:]))

    LAST_IN_MAPS = in_maps
    nc = _get_nc()
    res = bass_utils.run_bass_kernel_spmd(
        nc, in_maps, core_ids=list(range(NCORES)), trace=TRACE
    )
    LAST_RESULTS = res

    out = np.empty((B, C), dtype=np.float32)
    inv_scale = np.float32(1.0 / WSCALE)
    for t in range(T):
        seg_idx = order[starts[t] : starts[t + 1]]
        n_dev = min(counts[t], NCAP)
        # out_d[g, 32j+c, 512q+r] -> row 2048*(4g+q) + 512*j + r
        o = np.asarray(res.results[t]["out"]).astype(np.float32)
        o = o.reshape(NG, 4, C, GQ, CH).transpose(0, 3, 1, 4, 2).reshape(NCAP, C)
        out[seg_idx[:n_dev]] = o[:n_dev]
    out *= inv_scale
    for t, idx in over_rows:
        out[idx] = x[idx] @ W[t].T
    out += b[labels]
    return out


# revision 24
# speedup vs baseline: 1.0918x; 1.0208x over previous
"""Trainium2 kernel for nn_MultiHeadClassifier.

Math: out[i] = W[task_labels[i]] @ x[i] + b[task_labels[i]]
  x [262144, 1024] f32, task_labels [262144] int, W [8, 32, 1024], b [8, 32]

Strategy (8 NeuronCores, task-parallel, fp8 x):
  - Host sorts rows by task; core c processes (up to NCAP=32768) rows of
    task c, so W[c] is a per-core constant and there is NO routing on
    device at all — each core runs a plain GEMM. The ~few hundred rows
    that overflow a core's capacity are computed on host (numpy) and the
    result is merged back; bias is added on host.
  - x is sent as fp8 e3m4 (1 byte: 4 mantissa bits), quartering the
    dominant HBM traffic vs f32. W is scaled by 128 and sent as an
    e3m4 hi+lo pair, which cancels the W quantization error to second
    order; the host divides the output by 128 (exact). Measured rel
    err 1.16e-2 vs the 2e-2 gate.
  - hi and lo live side by side in one M=64 stationary [128, 64], so
    each k-tile needs ONE matmul; the hi+lo reduction happens on the
    (otherwise idle) DVE as a fused PSUM add + bf16 cast. Two 512-row
    chunks pack into each PSUM bank via column tiling (positions 0/64),
    which the PE streams 2-way concurrently (the XBUS ceiling measured
    on HW) -> ~3.4us of PE per 2048-row quad vs ~5.2us of DMA.
  - x quads (2 MB, 16 KB/partition contiguous) stream on both HWDGE
    queues (SP/ACT alternating); wt is pre-transposed on host so its
    const DMA is a handful of contiguous descriptors (a host-side
    rearrange here costs 14 us of pipeline head). PSUM -> DVE add+cast
    to bf16 -> 512 KB output DMA per 4 quads (final group in 4 small
    pieces to cut the tail). Expected ~100-105 us.
"""

import sys

sys.path.insert(0, "/opt/trn_rl_repo")

import numpy as np
import ml_dtypes

import concourse.bass as bass
import concourse.tile as tile
from concourse import bacc, mybir
from concourse import bass_utils

B, D, C, T = 262144, 1024, 32, 8
NCORES = 8
P = 128
KO = D // P  # 8 contraction k-tiles
CH = 512  # rows per chunk (one PSUM column-tile)
QR = 2048  # rows per quad (one x DMA; 4 chunks in 2 PSUM banks)
NQ = 16  # quads per core
NCAP = QR * NQ  # 32768 rows per core capacity
GQ = 4  # quads per output DMA group
NG = NQ // GQ  # output groups
WSCALE = 128.0  # power of two; host divides the output by this exactly

F8 = ml_dtypes.float8_e3m4
F8LIM = 15.5  # max finite e3m4

# set by test harness to collect a profile; harness-invoked kernel() keeps it off
TRACE = False
LAST_RESULTS = None
LAST_IN_MAPS = None


def _build():
    f32 = mybir.dt.float32
    bf16 = mybir.dt.bfloat16
    f8 = mybir.dt.float8e3

    nc = bacc.Bacc("TRN2", debug=False, num_devices=NCORES)
    # xt[m, ki, ko, r]: one quad is a contiguous 2 MB region with
    # 16 KB contiguous per partition.
    xt_d = nc.dram_tensor("xt", [NQ, P, KO, QR], f8, kind="ExternalInput")
    # wt[ki, ko, 0:32]=hi, [ki, ko, 32:64]=lo (host-transposed, scaled)
    wt_d = nc.dram_tensor("wt", [P, KO, 2 * C], f8, kind="ExternalInput")
    out_d = nc.dram_tensor("out", [NG, P, GQ * CH], bf16, kind="ExternalOutput")
    # final quad ships its two PSUM banks unmerged; host adds hi+lo
    outz_d = nc.dram_tensor("outz", [2, P, CH], bf16, kind="ExternalOutput")

    with tile.TileContext(nc) as tc:
        with (
            tc.tile_pool(name="consts", bufs=1) as consts,
            tc.tile_pool(name="xpool", bufs=6) as xpool,
            tc.tile_pool(name="opool", bufs=3) as opool,
            tc.tile_pool(name="psum", bufs=6, space="PSUM") as psum,
        ):
            # first x quad in flight before the consts
            xq0 = xpool.tile([P, KO, QR], f8, tag="xq")
            nc.sync.dma_start(xq0[:], xt_d[0])

            # consts on the ACT ring (contiguous layout: cheap descriptors)
            wt = consts.tile([P, KO, 2 * C], f8)
            nc.scalar.dma_start(wt[:], wt_d[:])

            # Engine warmups: give PE and DVE one instruction that observes
            # the const DMA lane so steady-state instructions carry at most
            # one sync wait each.
            scratch = psum.tile([P, CH], f32, tag="y4")
            nc.tensor.matmul(
                scratch[:2, :2], wt[:, 0, :2], wt[:, 0, :2], start=True, stop=True
            )
            dve_scr = consts.tile([1, 2 * C], bf16)
            nc.vector.tensor_copy(dve_scr[:], wt[:1, 0, :])

            for m in range(NQ):
                g, q = m // GQ, m % GQ
                last_g = g == NG - 1
                if m == 0:
                    xq = xq0
                else:
                    xq = xpool.tile([P, KO, QR], f8, tag="xq")
                    # all x on the SP ring: the ACT sequencer is busy with
                    # hi-copies, and x triggers must never queue behind them
                    nc.sync.dma_start(xq[:], xt_d[m])
                if q == 0 and not last_g:
                    out_g = opool.tile([P, GQ * CH], bf16, tag="out")
                # 2 banks x 2 column positions = 4 chunks of 512 rows
                ya = psum.tile([P, CH], f32, tag="y4")
                yb = psum.tile([P, CH], f32, tag="y4")
                for ko in range(KO):
                    for bank, (j0, j1) in ((ya, (0, 1)), (yb, (2, 3))):
                        for pos, j in ((0, j0), (64, j1)):
                            nc.tensor.matmul(
                                bank[pos : pos + 2 * C, :],
                                wt[:, ko, :],
                                xq[:, ko, CH * j : CH * (j + 1)],
                                start=(ko == 0),
                                stop=(ko == KO - 1),
                                tile_position=(0, pos),
                                skip_group_check=True,
                            )
                if m == NQ - 1:
                    # final quad: no on-device merge — cast both banks in
                    # parallel (ACT + DVE) and ship raw; host adds hi+lo
                    za = opool.tile([P, CH], bf16, tag="piece")
                    zb = opool.tile([P, CH], bf16, tag="piece")
                    nc.scalar.copy(za[:], ya[:])
                    nc.vector.tensor_copy(zb[:], yb[:])
                    nc.scalar.dma_start(outz_d[0], za[:])
                    nc.scalar.dma_start(outz_d[1], zb[:])
                    continue
                if last_g:
                    # final group: small pieces so the tail DMAs after the
                    # last matmuls are ~128 KB, not 512 KB
                    piece = opool.tile([P, CH], bf16, tag="piece")
                    dst = piece
                    off = 0
                else:
                    dst = out_g
                    off = CH * q
                for j in range(4):
                    bank = ya if j < 2 else yb
                    base = 64 * (j % 2)
                    sl = dst[C * j : C * (j + 1), off : off + CH]
                    # out = hi + lo: ACT copies hi w/ bf16 cast, DVE adds lo
                    # (an instruction may read at most one PSUM operand)
                    nc.scalar.copy(sl, bank[base : base + C, :])
                    nc.vector.tensor_tensor(
                        sl, sl, bank[base + C : base + 2 * C, :], mybir.AluOpType.add
                    )
                if last_g:
                    nc.scalar.dma_start(out_d[g, :, CH * q : CH * (q + 1)], piece[:])
                elif q == GQ - 1:
                    nc.scalar.dma_start(out_d[g], out_g[:])
    nc.compile()
    return nc


_NC = None


def _get_nc():
    global _NC
    if _NC is None:
        _NC = _build()
    return _NC


def kernel(x, task_labels, W, b):
    global LAST_RESULTS, LAST_IN_MAPS
    x = np.asarray(x)
    if x.dtype != np.float32:
        x = x.astype(np.float32)
    labels = np.asarray(task_labels).astype(np.int64)
    W = np.asarray(W)
    if W.dtype != np.float32:
        W = W.astype(np.float32)
    b = np.asarray(b)
    if b.dtype != np.float32:
        b = b.astype(np.float32)

    order = np.argsort(labels, kind="stable")  # rows grouped by task
    counts = np.bincount(labels, minlength=T)
    starts = np.concatenate([[0], np.cumsum(counts)])

    in_maps = []
    over_rows = []  # (task, global row indices beyond capacity)
    for t in range(T):
        seg_idx = order[starts[t] : starts[t + 1]]
        n_dev = min(counts[t], NCAP)
        xs = np.zeros((NCAP, D), dtype=F8)
        xs[:n_dev] = x[seg_idx[:n_dev]]
        # xt[m, ki, ko, r] = xs[m*QR + r, ko*P + ki]
        xt = np.ascontiguousarray(xs.reshape(NQ, QR, KO, P).transpose(0, 3, 2, 1))
        ws = W[t].astype(np.float64) * WSCALE
        hi = np.clip(ws, -F8LIM, F8LIM).astype(F8)
        lo = np.clip(ws - hi.astype(np.float64), -F8LIM, F8LIM).astype(F8)
        # wt[ki, ko, 0:32]=hi[c, ko*128+ki], [ki, ko, 32:64]=lo
        wt = np.empty((P, KO, 2 * C), dtype=F8)
        wt[:, :, :C] = hi.T.reshape(KO, P, C).transpose(1, 0, 2)
        wt[:, :, C:] = lo.T.reshape(KO, P, C).transpose(1, 0, 2)
        in_maps.append({"xt": xt, "wt": np.ascontiguousarray(wt)})
        if counts[t] > NCAP:
            over_rows.append((t, seg_idx[NCAP:]))

    LAST_IN_MAPS = in_maps
    nc = _get_nc()
    res = bass_utils.run_bass_kernel_spmd(
        nc, in_maps, core_ids=list(range(NCORES)), trace=TRACE
    )
    LAST_RESULTS = res

    out = np.empty((B, C), dtype=np.float32)
    inv_scale = np.float32(1.0 / WSCALE)
    for t in range(T):
        seg_idx = order[starts[t] : starts[t + 1]]
        n_dev = min(counts[t], NCAP)
        # out_d[g, 32j+c, 512q+r] -> row 2048*(4g+q) + 512*j + r
        o = np.asarray(res.results[t]["out"]).astype(np.float32)
        o = o.reshape(NG, 4, C, GQ, CH).transpose(0, 3, 1, 4, 2).reshape(NCAP, C)
        # final quad arrives unmerged: oz[bank, 64*jj+{0:32 hi, 32:64 lo}, r]
        oz = np.asarray(res.results[t]["outz"]).astype(np.float32)
        oz = oz.reshape(2, 2, 2, C, CH)  # [bank, jj, hi/lo, c, r]
        zm = (oz[:, :, 0] + oz[:, :, 1]).transpose(0, 1, 3, 2)  # [bank, jj, r, c]
        o[NCAP - 4 * CH :] = zm.reshape(4 * CH, C)
        out[seg_idx[:n_dev]] = o[:n_dev]
    out *= inv_scale
    for t, idx in over_rows:
        out[idx] = x[idx] @ W[t].T
    out += b[labels]
    return out
